# revision 23
# baseline (speedup 1.0000x reference)
"""Causal self-attention (B=4, S=2048, E=1024, H=16) on 8 TRN2 NeuronCores.

Sharding: core = (batch b, head-group g): b = core // 2, g = core % 2.
Each core handles one batch and 8 of the 16 heads (Megatron-style column
parallel QKV + row-parallel out-proj); the two half-projections per batch
are summed on the host.

All weights/activations are pre-transposed and cast to bf16 on the host so
the on-chip program is pure matmul + softmax:
  qT/kT [j=h*64+d, s] = WT.T @ xT        (heads on partitions)
  v     [s, j]        = xT.T @ WvT       (natural layout, + ones column)
  scT   [kj, qi]      = kT_h.T' @ qT_h   (K=64, 2 heads row-packed in PE)
  e = exp(0.125*scT) * causal_mask       (ScalarE from PSUM, DVE mask)
  pv    [65, qi]      = [v_h | 1].T @ e  (accumulated over kj; row 64 = rowsum)
  o     = pv[0:64] / pv[64]              (DVE fast recip + GpSimd part-bcast)
  outT  [e, s]        = WpT.T @ o_cat    (partial; host sums the two groups)

The attention inner loop is ScalarE(exp)-bound, so the QKV / V / out-proj
matmul streams are WOVEN into the attention emission order: the PE gets
filler work during exp-gated stretches, which also keeps the HAM clock
un-throttled. Upper-triangle key tiles are skipped; diagonal key tiles
compute only the valid column range [128r:512).
"""

import sys

for _p in ("/opt/trn_rl_repo", "/root/.axon_site/_ro/trn_rl_repo"):
    if _p not in sys.path:
        sys.path.append(_p)

from contextlib import ExitStack

import numpy as np
import ml_dtypes

import concourse.bass as bass
import concourse.tile as tile
import concourse.mybir as mybir
from concourse import bacc
from concourse.bass_utils import run_bass_kernel_spmd

BF16 = mybir.dt.bfloat16
F32 = mybir.dt.float32
NP_BF16 = ml_dtypes.bfloat16

B, S, E, H = 4, 2048, 1024, 16
D = E // H            # 64
HL = H // 2           # 8 heads per core
JC = HL * D           # 512 local head-concat width
P = 128
NKT = S // P          # 16 key tiles
NQT = S // 512        # 4 query tiles of 512
EKT = E // P          # 8 contraction tiles for QKV projections
CT = JC // P          # 4 contraction tiles for the output projection
SCALE = 1.0 / np.sqrt(np.float32(D))  # 0.125


def build_program(apply_key_mask: bool):
    nc = bacc.Bacc("TRN2", target_bir_lowering=False, debug=False, num_devices=8)

    xT = nc.dram_tensor("xT", [E, S], BF16, kind="ExternalInput").ap()
    wqT = nc.dram_tensor("wqT", [E, JC], BF16, kind="ExternalInput").ap()
    wkT = nc.dram_tensor("wkT", [E, JC], BF16, kind="ExternalInput").ap()
    wvT = nc.dram_tensor("wvT", [E, JC], BF16, kind="ExternalInput").ap()
    wpT = nc.dram_tensor("wpT", [JC, E], BF16, kind="ExternalInput").ap()
    cmask = nc.dram_tensor("cmask", [4, P, 512], BF16, kind="ExternalInput").ap()
    if apply_key_mask:
        kmaskT = nc.dram_tensor("kmaskT", [P, NKT], F32, kind="ExternalInput").ap()
    outp = nc.dram_tensor("outp", [E, S], F32, kind="ExternalOutput").ap()

    xT_r = xT.rearrange("(kt p) s -> p kt s", p=P)
    wq_r = wqT.rearrange("(kt p) j -> p kt j", p=P)
    wk_r = wkT.rearrange("(kt p) j -> p kt j", p=P)
    wv_r = wvT.rearrange("(kt p) j -> p kt j", p=P)

    with tile.TileContext(nc) as tc:
        with ExitStack() as ctx:
            per = ctx.enter_context(tc.tile_pool(name="per", bufs=1))
            sc_ps = ctx.enter_context(
                tc.tile_pool(name="sc_ps", bufs=2, space="PSUM")
            )
            pv_ps = ctx.enter_context(
                tc.tile_pool(name="pv_ps", bufs=3, space="PSUM")
            )
            fill_ps = ctx.enter_context(
                tc.tile_pool(name="fill_ps", bufs=1, space="PSUM")
            )
            esb = ctx.enter_context(tc.tile_pool(name="esb", bufs=3))
            nrm = ctx.enter_context(tc.tile_pool(name="nrm", bufs=4))
            posb = ctx.enter_context(tc.tile_pool(name="posb", bufs=3))

            # spread the big input loads across independent DGE queues
            wq_sb = per.tile([P, EKT, JC], BF16, tag="wq")
            wk_sb = per.tile([P, EKT, JC], BF16, tag="wk")
            wv_sb = per.tile([P, EKT, JC], BF16, tag="wv")
            xT_sb = per.tile([P, EKT, S], BF16, tag="xT")
            # x gates the first QKV block: give it two dedicated queues,
            # weights stream on the third
            for kt in range(EKT):
                (nc.sync if kt % 2 == 0 else nc.gpsimd).dma_start(
                    xT_sb[:, kt], xT_r[:, kt]
                )
            for kt in range(EKT):
                nc.scalar.dma_start(wq_sb[:, kt], wq_r[:, kt])
            for kt in range(EKT):
                nc.scalar.dma_start(wk_sb[:, kt], wk_r[:, kt])
            for kt in range(EKT):
                nc.scalar.dma_start(wv_sb[:, kt], wv_r[:, kt])
            wp_sb = per.tile([P, CT, E], BF16, tag="wp")
            nc.scalar.dma_start(wp_sb[:], wpT.rearrange("(ct p) e -> p ct e", p=P))
            cm_sb = per.tile([P, 4, 512], BF16, tag="cm")
            nc.scalar.dma_start(cm_sb[:], cmask.rearrange("r p c -> p r c"))
            if apply_key_mask:
                km_sb = per.tile([P, NKT], F32, tag="km")
                nc.sync.dma_start(km_sb[:], kmaskT[:])

            qT_sb = per.tile([P, CT, S], BF16, tag="qT")
            kT_sb = per.tile([P, CT, S], BF16, tag="kT")
            vaug_sb = per.tile([P, NKT, HL, D + 1], BF16, tag="vaug")
            o_sb = per.tile([P, CT, S], BF16, tag="o")

            nc.vector.memset(vaug_sb[:, :, :, D], 1.0)

            # preload the exp table set on ScalarE while DMAs stream in, so
            # the ~2.7us ACT_TABLE_LOAD is off the attention critical path
            warm = nrm.tile([1, 16], F32, tag="warm")
            nc.vector.memset(warm[:], 0.0)
            warm2 = nrm.tile([1, 16], F32, tag="warm2")
            nc.scalar.activation(
                warm2[:], warm[:], mybir.ActivationFunctionType.Exp
            )

            def emit_qk(jt):
                for w_sb, dst in ((wq_sb, qT_sb), (wk_sb, kT_sb)):
                    for st in range(NQT):
                        ps = fill_ps.tile([P, 512], F32, tag="ps")
                        for kt in range(EKT):
                            nc.tensor.matmul(
                                ps[:],
                                w_sb[:, kt, jt * P : (jt + 1) * P],
                                xT_sb[:, kt, st * 512 : (st + 1) * 512],
                                start=(kt == 0),
                                stop=(kt == EKT - 1),
                            )
                        nc.vector.tensor_copy(
                            dst[:, jt, st * 512 : (st + 1) * 512], ps[:]
                        )

            def emit_v(st_lo, st_hi):
                for st in range(st_lo, st_hi):
                    ps = fill_ps.tile([P, 512], F32, tag="ps")
                    for kt in range(EKT):
                        nc.tensor.matmul(
                            ps[:],
                            xT_sb[:, kt, st * P : (st + 1) * P],
                            wv_sb[:, kt, :],
                            start=(kt == 0),
                            stop=(kt == EKT - 1),
                        )
                    nc.vector.tensor_copy(
                        vaug_sb[:, st, :, 0:D],
                        ps[:].rearrange("p (h d) -> p h d", d=D),
                    )
                    if apply_key_mask:
                        nc.vector.tensor_scalar_mul(
                            vaug_sb[:, st], vaug_sb[:, st], km_sb[:, st : st + 1]
                        )

            def emit_attn(qt, a):
                qs0 = qt * 512
                pv = pv_ps.tile([P, 512], F32, tag="pv")
                pv2 = pv_ps.tile([P, 512], F32, tag="pv")
                nkt = 4 * qt + 4
                for kt in range(nkt):
                    r = kt - 4 * qt
                    c0 = 128 * r if r > 0 else 0  # first valid column
                    ks = slice(kt * P, (kt + 1) * P)
                    qs = slice(qs0 + c0, qs0 + 512)
                    sc = sc_ps.tile([P, 1024], F32, tag="sc")
                    nc.tensor.matmul(
                        sc[:, c0:512],
                        kT_sb[0:D, a, ks],
                        qT_sb[0:D, a, qs],
                        start=True,
                        stop=True,
                    )
                    nc.tensor.matmul(
                        sc[:, 512 + c0 : 1024],
                        kT_sb[D : 2 * D, a, ks],
                        qT_sb[D : 2 * D, a, qs],
                        start=True,
                        stop=True,
                    )
                    e = esb.tile([P, 1024], BF16, tag="e")
                    e2 = e[:].rearrange("p (two c) -> p two c", two=2)
                    sc2 = sc[:].rearrange("p (two c) -> p two c", two=2)
                    nc.scalar.activation(
                        e2[:, :, c0:512], sc2[:, :, c0:512],
                        mybir.ActivationFunctionType.Exp,
                        scale=float(SCALE),
                    )
                    if r >= 0:
                        # causal mask: only the first 128-col subblock of the
                        # valid range is partially masked (triangular); the
                        # rest is fully valid
                        nc.vector.tensor_mul(
                            e2[:, :, c0 : c0 + 128],
                            e2[:, :, c0 : c0 + 128],
                            cm_sb[:, 0, None, 0:128].to_broadcast((P, 2, 128)),
                        )
                    nc.tensor.matmul(
                        pv[0 : D + 1, c0:512],
                        vaug_sb[:, kt, 2 * a, :],
                        e[:, c0:512],
                        start=(kt == 0),
                        stop=(kt == nkt - 1),
                        skip_group_check=True,
                    )
                    nc.tensor.matmul(
                        pv2[0 : D + 1, c0:512],
                        vaug_sb[:, kt, 2 * a + 1, :],
                        e[:, 512 + c0 : 1024],
                        start=(kt == 0),
                        stop=(kt == nkt - 1),
                        skip_group_check=True,
                    )
                # normalize: o = pv[0:64] * (1 / pv[64])
                qsl = slice(qs0, qs0 + 512)
                for h_par, pvt in ((0, pv), (1, pv2)):
                    # custom-DVE recip misreads PSUM operands: stage in SBUF
                    rsum = nrm.tile([1, 512], F32, tag="rsum")
                    nc.vector.tensor_copy(rsum[:], pvt[D : D + 1, :])
                    rec = nrm.tile([1, 512], F32, tag="rec")
                    nc.vector.reciprocal_approx_fast(rec[:], rsum[:])
                    bc = nrm.tile([D, 512], F32, tag="bc")
                    nc.gpsimd.partition_broadcast(bc[:], rec[:])
                    if h_par == 0:
                        nc.vector.tensor_mul(o_sb[0:D, a, qsl], pvt[0:D, :], bc[:])
                    else:
                        tmp = nrm.tile([D, 512], BF16, tag="tmp")
                        nc.vector.tensor_mul(tmp[:], pvt[0:D, :], bc[:])
                        # shift to partitions 64..127 (DVE can't)
                        nc.gpsimd.dma_start(o_sb[D : 2 * D, a, qsl], tmp[:])

            def emit_proj(st, et_lo, et_hi):
                for et in range(et_lo, et_hi):
                    ps = fill_ps.tile([P, 512], F32, tag="ps")
                    for ct in range(CT):
                        nc.tensor.matmul(
                            ps[:],
                            wp_sb[:, ct, et * P : (et + 1) * P],
                            o_sb[:, ct, st * 512 : (st + 1) * 512],
                            start=(ct == 0),
                            stop=(ct == CT - 1),
                        )
                    po = posb.tile([P, 512], F32, tag="po")
                    nc.vector.tensor_copy(po[:], ps[:])
                    nc.sync.dma_start(
                        outp[et * P : (et + 1) * P, st * 512 : (st + 1) * 512],
                        po[:],
                    )

            # Interleaved schedule: attention's exp stream (ScalarE) overlaps
            # the QKV / V / proj matmul streams (PE filler → HAM stays warm).
            emit_qk(0)
            emit_v(0, 4)
            emit_attn(0, 0)
            emit_v(4, 8)
            emit_attn(1, 0)
            emit_qk(1)
            emit_attn(0, 1)
            emit_attn(1, 1)
            emit_qk(2)
            emit_attn(0, 2)
            emit_attn(1, 2)
            emit_qk(3)
            emit_attn(0, 3)
            emit_attn(1, 3)

            emit_v(8, 16)
            emit_attn(3, 0)
            emit_proj(0, 0, 4)
            emit_attn(3, 1)
            emit_proj(0, 4, 8)
            emit_attn(3, 2)
            emit_proj(1, 0, 4)
            emit_attn(3, 3)
            emit_proj(1, 4, 8)

            emit_attn(2, 0)
            emit_proj(3, 0, 4)
            emit_attn(2, 1)
            emit_proj(3, 4, 8)
            emit_attn(2, 2)
            emit_attn(2, 3)
            emit_proj(2, 0, 8)

    nc.compile()
    return nc


def _causal_masks() -> np.ndarray:
    p = np.arange(P)[:, None]
    c = np.arange(512)[None, :]
    m = np.stack([(c >= p + P * r) for r in range(4)]).astype(np.float32)
    return m.astype(NP_BF16)


def kernel(input, attention_mask, Wq, Wk, Wv, Wp, _profile=False):
    input = np.asarray(input, dtype=np.float32)
    attention_mask = np.asarray(attention_mask)
    Wq, Wk, Wv, Wp = (np.asarray(w, dtype=np.float32) for w in (Wq, Wk, Wv, Wp))

    mask_all = bool(attention_mask.all())
    nc = build_program(apply_key_mask=not mask_all)

    cm = _causal_masks()
    in_maps = []
    for core in range(8):
        b, g = core // 2, core % 2
        rows = slice(g * JC, (g + 1) * JC)
        m = {
            "xT": np.ascontiguousarray(input[b].T).astype(NP_BF16),
            "wqT": np.ascontiguousarray(Wq[rows].T).astype(NP_BF16),
            "wkT": np.ascontiguousarray(Wk[rows].T).astype(NP_BF16),
            "wvT": np.ascontiguousarray(Wv[rows].T).astype(NP_BF16),
            "wpT": np.ascontiguousarray(Wp[:, rows].T).astype(NP_BF16),
            "cmask": cm,
        }
        if not mask_all:
            km = attention_mask[b].astype(np.float32)  # [S]
            m["kmaskT"] = np.ascontiguousarray(km.reshape(NKT, P).T)
        in_maps.append(m)

    res = run_bass_kernel_spmd(
        nc, in_maps, core_ids=list(range(8)), trace=_profile
    )

    out = np.empty((B, S, E), dtype=np.float32)
    for b in range(B):
        acc = res.results[2 * b]["outp"] + res.results[2 * b + 1]["outp"]
        out[b] = acc.T
    if _profile:
        return out, res
    return out


# revision 24
# speedup vs baseline: 1.1185x; 1.1185x over previous
"""Causal self-attention (B=4, S=2048, E=1024, H=16) on 8 TRN2 NeuronCores.

Sharding: core = (batch b, head-group g): b = core // 2, g = core % 2.
Each core handles one batch and 8 of the 16 heads (Megatron-style column
parallel QKV + row-parallel out-proj); the two half-projections per batch
are summed on the host.

All weights/activations are pre-transposed and cast to bf16 on the host so
the on-chip program is pure matmul + softmax:
  qT/kT [j=h*64+d, s] = WT.T @ xT        (heads on partitions)
  v     [s, j]        = xT.T @ WvT       (natural layout, + ones column)
  scT   [kj, qi]      = kT_h.T' @ qT_h   (K=64, 2 heads row-packed in PE)
  e = exp(0.125*scT) * causal_mask       (ScalarE from PSUM, DVE mask)
  pv    [65, qi]      = [v_h | 1].T @ e  (accumulated over kj; row 64 = rowsum)
  o     = pv[0:64] / pv[64]              (DVE fast recip + GpSimd part-bcast)
  outT  [e, s]        = WpT.T @ o_cat    (partial; host sums the two groups)

The attention inner loop is ScalarE(exp)-bound, so the QKV / V / out-proj
matmul streams are WOVEN into the attention emission order: the PE gets
filler work during exp-gated stretches, which also keeps the HAM clock
un-throttled. Upper-triangle key tiles are skipped; diagonal key tiles
compute only the valid column range [128r:512).
"""

import sys

for _p in ("/opt/trn_rl_repo", "/root/.axon_site/_ro/trn_rl_repo"):
    if _p not in sys.path:
        sys.path.append(_p)

from contextlib import ExitStack

import numpy as np
import ml_dtypes

import concourse.bass as bass
import concourse.tile as tile
import concourse.mybir as mybir
from concourse import bacc
from concourse.bass_utils import run_bass_kernel_spmd

BF16 = mybir.dt.bfloat16
F32 = mybir.dt.float32
NP_BF16 = ml_dtypes.bfloat16

B, S, E, H = 4, 2048, 1024, 16
D = E // H            # 64
HL = H // 2           # 8 heads per core
JC = HL * D           # 512 local head-concat width
P = 128
NKT = S // P          # 16 key tiles
NQT = S // 512        # 4 query tiles of 512
EKT = E // P          # 8 contraction tiles for QKV projections
CT = JC // P          # 4 contraction tiles for the output projection
SCALE = 1.0 / np.sqrt(np.float32(D))  # 0.125


def build_program(apply_key_mask: bool):
    nc = bacc.Bacc("TRN2", target_bir_lowering=False, debug=False, num_devices=8)

    xT = nc.dram_tensor("xT", [E, S], BF16, kind="ExternalInput").ap()
    wqT = nc.dram_tensor("wqT", [E, JC], BF16, kind="ExternalInput").ap()
    wkT = nc.dram_tensor("wkT", [E, JC], BF16, kind="ExternalInput").ap()
    wvT = nc.dram_tensor("wvT", [E, JC], BF16, kind="ExternalInput").ap()
    wpT = nc.dram_tensor("wpT", [JC, E], BF16, kind="ExternalInput").ap()
    cmask = nc.dram_tensor("cmask", [4, P, 512], BF16, kind="ExternalInput").ap()
    if apply_key_mask:
        kmaskT = nc.dram_tensor("kmaskT", [P, NKT], F32, kind="ExternalInput").ap()
    outp = nc.dram_tensor("outp", [E, S], F32, kind="ExternalOutput").ap()

    xT_r = xT.rearrange("(kt p) s -> p kt s", p=P)
    wq_r = wqT.rearrange("(kt p) j -> p kt j", p=P)
    wk_r = wkT.rearrange("(kt p) j -> p kt j", p=P)
    wv_r = wvT.rearrange("(kt p) j -> p kt j", p=P)

    with tile.TileContext(nc) as tc:
        with ExitStack() as ctx:
            per = ctx.enter_context(tc.tile_pool(name="per", bufs=1))
            sc_ps = ctx.enter_context(
                tc.tile_pool(name="sc_ps", bufs=2, space="PSUM")
            )
            pv_ps = ctx.enter_context(
                tc.tile_pool(name="pv_ps", bufs=2, space="PSUM")
            )
            fill_ps = ctx.enter_context(
                tc.tile_pool(name="fill_ps", bufs=2, space="PSUM")
            )
            esb = ctx.enter_context(tc.tile_pool(name="esb", bufs=3))
            nrm = ctx.enter_context(tc.tile_pool(name="nrm", bufs=4))
            posb = ctx.enter_context(tc.tile_pool(name="posb", bufs=3))

            # spread the big input loads across independent DGE queues
            wq_sb = per.tile([P, EKT, JC], BF16, tag="wq")
            wk_sb = per.tile([P, EKT, JC], BF16, tag="wk")
            wv_sb = per.tile([P, EKT, JC], BF16, tag="wv")
            xT_sb = per.tile([P, EKT, S], BF16, tag="xT")
            # x gates the first QKV block: give it two dedicated queues,
            # weights stream on the third
            for kt in range(EKT):
                (nc.sync if kt % 2 == 0 else nc.gpsimd).dma_start(
                    xT_sb[:, kt], xT_r[:, kt]
                )
            for kt in range(EKT):
                nc.scalar.dma_start(wq_sb[:, kt], wq_r[:, kt])
            for kt in range(EKT):
                nc.scalar.dma_start(wk_sb[:, kt], wk_r[:, kt])
            for kt in range(EKT):
                nc.scalar.dma_start(wv_sb[:, kt], wv_r[:, kt])
            wp_sb = per.tile([P, CT, E], BF16, tag="wp")
            nc.scalar.dma_start(wp_sb[:], wpT.rearrange("(ct p) e -> p ct e", p=P))
            cm_sb = per.tile([P, 4, 512], BF16, tag="cm")
            nc.scalar.dma_start(cm_sb[:], cmask.rearrange("r p c -> p r c"))
            if apply_key_mask:
                km_sb = per.tile([P, NKT], F32, tag="km")
                nc.sync.dma_start(km_sb[:], kmaskT[:])

            qT_sb = per.tile([P, CT, S], BF16, tag="qT")
            kT_sb = per.tile([P, CT, S], BF16, tag="kT")
            vaug_sb = per.tile([P, NKT, HL, D + 1], BF16, tag="vaug")
            o_sb = per.tile([P, CT, S], BF16, tag="o")

            nc.vector.memset(vaug_sb[:, :, :, D], 1.0)

            # preload the exp table set on ScalarE while DMAs stream in, so
            # the ~2.7us ACT_TABLE_LOAD is off the attention critical path
            warm = nrm.tile([1, 16], F32, tag="warm")
            nc.vector.memset(warm[:], 0.0)
            warm2 = nrm.tile([1, 16], F32, tag="warm2")
            nc.scalar.activation(
                warm2[:], warm[:], mybir.ActivationFunctionType.Exp
            )

            def emit_qk(jt):
                for w_sb, dst in ((wq_sb, qT_sb), (wk_sb, kT_sb)):
                    for st in range(NQT):
                        ps = fill_ps.tile([P, 512], F32, tag="ps")
                        for kt in range(EKT):
                            nc.tensor.matmul(
                                ps[:],
                                w_sb[:, kt, jt * P : (jt + 1) * P],
                                xT_sb[:, kt, st * 512 : (st + 1) * 512],
                                start=(kt == 0),
                                stop=(kt == EKT - 1),
                            )
                        nc.vector.tensor_copy(
                            dst[:, jt, st * 512 : (st + 1) * 512], ps[:]
                        )

            def emit_v(st_lo, st_hi):
                for st in range(st_lo, st_hi):
                    ps = fill_ps.tile([P, 512], F32, tag="ps")
                    for kt in range(EKT):
                        nc.tensor.matmul(
                            ps[:],
                            xT_sb[:, kt, st * P : (st + 1) * P],
                            wv_sb[:, kt, :],
                            start=(kt == 0),
                            stop=(kt == EKT - 1),
                        )
                    nc.vector.tensor_copy(
                        vaug_sb[:, st, :, 0:D],
                        ps[:].rearrange("p (h d) -> p h d", d=D),
                    )
                    if apply_key_mask:
                        nc.vector.tensor_scalar_mul(
                            vaug_sb[:, st], vaug_sb[:, st], km_sb[:, st : st + 1]
                        )

            def emit_attn(qt, a):
                qs0 = qt * 512
                pv = pv_ps.tile([P, 512], F32, tag="pv")
                pv2 = pv_ps.tile([P, 512], F32, tag="pv")
                nkt = 4 * qt + 4
                for kt in range(nkt):
                    r = kt - 4 * qt
                    c0 = 128 * r if r > 0 else 0  # first valid column
                    ks = slice(kt * P, (kt + 1) * P)
                    qs = slice(qs0 + c0, qs0 + 512)
                    sc = sc_ps.tile([P, 1024], F32, tag="sc")
                    nc.tensor.matmul(
                        sc[:, c0:512],
                        kT_sb[0:D, a, ks],
                        qT_sb[0:D, a, qs],
                        start=True,
                        stop=True,
                    )
                    nc.tensor.matmul(
                        sc[:, 512 + c0 : 1024],
                        kT_sb[D : 2 * D, a, ks],
                        qT_sb[D : 2 * D, a, qs],
                        start=True,
                        stop=True,
                    )
                    e = esb.tile([P, 1024], BF16, tag="e")
                    e2 = e[:].rearrange("p (two c) -> p two c", two=2)
                    sc2 = sc[:].rearrange("p (two c) -> p two c", two=2)
                    nc.scalar.activation(
                        e2[:, :, c0:512], sc2[:, :, c0:512],
                        mybir.ActivationFunctionType.Exp,
                        scale=float(SCALE),
                    )
                    if r >= 0:
                        # causal mask: only the first 128-col subblock of the
                        # valid range is partially masked (triangular); the
                        # rest is fully valid
                        nc.vector.tensor_mul(
                            e2[:, :, c0 : c0 + 128],
                            e2[:, :, c0 : c0 + 128],
                            cm_sb[:, 0, None, 0:128].to_broadcast((P, 2, 128)),
                        )
                    nc.tensor.matmul(
                        pv[0 : D + 1, c0:512],
                        vaug_sb[:, kt, 2 * a, :],
                        e[:, c0:512],
                        start=(kt == 0),
                        stop=(kt == nkt - 1),
                        skip_group_check=True,
                    )
                    nc.tensor.matmul(
                        pv2[0 : D + 1, c0:512],
                        vaug_sb[:, kt, 2 * a + 1, :],
                        e[:, 512 + c0 : 1024],
                        start=(kt == 0),
                        stop=(kt == nkt - 1),
                        skip_group_check=True,
                    )
                # normalize: o = pv[0:64] * (1 / pv[64])
                qsl = slice(qs0, qs0 + 512)
                for h_par, pvt in ((0, pv), (1, pv2)):
                    # custom-DVE recip misreads PSUM operands: stage in SBUF
                    rsum = nrm.tile([1, 512], F32, tag="rsum")
                    nc.vector.tensor_copy(rsum[:], pvt[D : D + 1, :])
                    rec = nrm.tile([1, 512], F32, tag="rec")
                    nc.vector.reciprocal_approx_fast(rec[:], rsum[:])
                    bc = nrm.tile([D, 512], F32, tag="bc")
                    nc.gpsimd.partition_broadcast(bc[:], rec[:])
                    if h_par == 0:
                        nc.vector.tensor_mul(o_sb[0:D, a, qsl], pvt[0:D, :], bc[:])
                    else:
                        tmp = nrm.tile([D, 512], BF16, tag="tmp")
                        nc.vector.tensor_mul(tmp[:], pvt[0:D, :], bc[:])
                        # shift to partitions 64..127 (DVE can't)
                        nc.gpsimd.dma_start(o_sb[D : 2 * D, a, qsl], tmp[:])

            def emit_proj(st, et_lo, et_hi):
                for et in range(et_lo, et_hi):
                    ps = fill_ps.tile([P, 512], F32, tag="ps")
                    for ct in range(CT):
                        nc.tensor.matmul(
                            ps[:],
                            wp_sb[:, ct, et * P : (et + 1) * P],
                            o_sb[:, ct, st * 512 : (st + 1) * 512],
                            start=(ct == 0),
                            stop=(ct == CT - 1),
                        )
                    po = posb.tile([P, 512], F32, tag="po")
                    nc.vector.tensor_copy(po[:], ps[:])
                    nc.sync.dma_start(
                        outp[et * P : (et + 1) * P, st * 512 : (st + 1) * 512],
                        po[:],
                    )

            # Interleaved schedule: attention's exp stream (ScalarE) overlaps
            # the QKV / V / proj matmul streams (PE filler → HAM stays warm).
            emit_qk(0)
            emit_v(0, 4)
            emit_attn(0, 0)
            emit_v(4, 8)
            emit_attn(1, 0)
            emit_qk(1)
            emit_attn(0, 1)
            emit_attn(1, 1)
            emit_qk(2)
            emit_attn(0, 2)
            emit_attn(1, 2)
            emit_qk(3)
            emit_attn(0, 3)
            emit_attn(1, 3)

            emit_v(8, 16)
            emit_attn(3, 0)
            emit_proj(0, 0, 4)
            emit_attn(3, 1)
            emit_proj(0, 4, 8)
            emit_attn(3, 2)
            emit_proj(1, 0, 4)
            emit_attn(3, 3)
            emit_proj(1, 4, 8)

            emit_attn(2, 0)
            emit_proj(3, 0, 4)
            emit_attn(2, 1)
            emit_proj(3, 4, 8)
            emit_attn(2, 2)
            emit_attn(2, 3)
            emit_proj(2, 0, 8)

    nc.compile()
    return nc


def _causal_masks() -> np.ndarray:
    p = np.arange(P)[:, None]
    c = np.arange(512)[None, :]
    m = np.stack([(c >= p + P * r) for r in range(4)]).astype(np.float32)
    return m.astype(NP_BF16)


def kernel(input, attention_mask, Wq, Wk, Wv, Wp, _profile=False):
    input = np.asarray(input, dtype=np.float32)
    attention_mask = np.asarray(attention_mask)
    Wq, Wk, Wv, Wp = (np.asarray(w, dtype=np.float32) for w in (Wq, Wk, Wv, Wp))

    mask_all = bool(attention_mask.all())
    nc = build_program(apply_key_mask=not mask_all)

    cm = _causal_masks()
    in_maps = []
    for core in range(8):
        b, g = core // 2, core % 2
        rows = slice(g * JC, (g + 1) * JC)
        m = {
            "xT": np.ascontiguousarray(input[b].T).astype(NP_BF16),
            "wqT": np.ascontiguousarray(Wq[rows].T).astype(NP_BF16),
            "wkT": np.ascontiguousarray(Wk[rows].T).astype(NP_BF16),
            "wvT": np.ascontiguousarray(Wv[rows].T).astype(NP_BF16),
            "wpT": np.ascontiguousarray(Wp[:, rows].T).astype(NP_BF16),
            "cmask": cm,
        }
        if not mask_all:
            km = attention_mask[b].astype(np.float32)  # [S]
            m["kmaskT"] = np.ascontiguousarray(km.reshape(NKT, P).T)
        in_maps.append(m)

    res = run_bass_kernel_spmd(
        nc, in_maps, core_ids=list(range(8)), trace=_profile
    )

    out = np.empty((B, S, E), dtype=np.float32)
    for b in range(B):
        acc = res.results[2 * b]["outp"] + res.results[2 * b + 1]["outp"]
        out[b] = acc.T
    if _profile:
        return out, res
    return out
